# revision 4
# baseline (speedup 1.0000x reference)
"""CLCE loss kernel for Trainium2 (8 NeuronCores, SPMD) — symmetric version.

Loss = 0.5 * cl + 0.5 * ce where
  cl_i = logsumexp(loss_temp_i) - slot0_i   over a [N, 2N-1] packed row
  ce   = cross-entropy of y_pred vs y_true.

Decomposition (exact, validated in f64 against the reference formula):
  cl_i = log(exp(slot0_i) + (T_i - P_i) + (2N-2 - num_neg_i)) - slot0_i
where
  T_i  = sum_j exp((xn_i . xn_j + 1) * 0.25)      <- the O(N^2 D) part, on device
  P_i  = sum_{j: y_j = y_i} exp(sim_ij)           <- O(N * class_size), on host
  slot0_i = sim_{i, first same-class j != i}      <- O(N), on host
  R_i  = sum_j exp(y_pred_ij)                     <- on device
  ce_i = log(R_i) - y_pred[i, y_i]

Symmetric device sharding (halves both matmul work and HBM traffic):
exp(sim) is symmetric, so T is both the row sums AND the column sums of it.
Split the 4096 rows into 16 blocks of 256.  Core c owns the two ADJACENT
row-blocks {2c, 2c+1} and a column stream rotated by 512*c.  Cover rule:
row-block r pairs with r+1..r+7 (mod 16), plus r+8 iff r is even, plus its
diagonal — every unordered pair lands exactly once globally.  In the rotated
stream that is rel col blocks 0..8 for slot0 (rows 2c) and 1..8 for slot1
(rows 2c+1): only 2304 distinct columns, and the stationary operand is the
stream's own first chunk (col blocks 0,1 = row blocks 2c,2c+1), so there is
no separate weight DMA.  Chunk 4 is packed as [rel block 1 | rel block 8] so
slot1's diagonal and d=7 pair form one clean 512-wide group.

Per chunk: fp8 DoubleRow matmuls accumulate dots in PSUM, ScalarE applies exp
(writing E as bf16), VectorE row-sums E (-> T of the row block).  Column sums
(-> T of the col block, via symmetry) are one-hot selector matmuls into a
persistent [16, 512] PSUM accumulator, DEFERRED to the end of the stream so
they never break the sim matmuls' LDWEIGHTS pipelining (all E tiles stay
resident in SBUF).  Diagonal 256-blocks are excluded from the colsums.  The
host adds rowsum/colsum/CE pieces and applies the same-class corrections in
full precision.
"""

import os
from contextlib import ExitStack

import numpy as np

import concourse.bass as bass
import concourse.tile as tile
from concourse import bacc, mybir
from concourse.bass_utils import run_bass_kernel_spmd

N, D, C = 4096, 1024, 512
TAU = 0.5
LAMBD = 0.5
NCORES = 8
P = 128                    # partitions
B = 256                    # row/col block granularity
NB = N // B                # 16 blocks
KT = D // 256              # 4 DoubleRow contraction super-tiles
W = 512                    # column-chunk width (1 psum bank)
NCH = 5                    # stream chunks (ch4 = [rel blk 1 | rel blk 8])
S8 = 16.0                  # fp8 pre-scale for the embeddings

_F32 = mybir.dt.float32
_BF16 = mybir.dt.bfloat16
_FP8 = mybir.dt.float8e4
_EXP = mybir.ActivationFunctionType.Exp
_DR = mybir.MatmulPerfMode.DoubleRow
_AXX = mybir.AxisListType.X


def _schedule():
    """(m, ch, rhs_off, width, cs_spans) in execution order.

    m 0,1 = rows block 2c (slot0); m 2,3 = rows block 2c+1 (slot1).
    cs_spans: (et_lo, et_hi, cs_row, cs_lo) selector-matmul specs, excluding
    the diagonal 256-blocks (slot0 rel blk 0 = ch0 h0; slot1 rel blk 1 =
    ch4 h0).  slot1 skips rel block 0 entirely (it is slot0's pair {2c,2c+1});
    its diagonal lives in ch4 h0.  CS row 4 cols 0:256 hold the rel-block-8
    colsums (stream cols 2048..2304).
    """
    sched = []
    for m in (0, 1):                        # ch0: slot0 only
        sched.append((m, 0, 0, 512, [(B, 2 * B, 0, B)]))
    for ch in (1, 2, 3):
        for m in range(4):
            sched.append((m, ch, 0, 512, [(0, 512, ch, 0)]))
    for m in (2, 3):                        # ch4: diag rb1 | pair rb8
        sched.append((m, 4, 0, 512, [(B, 2 * B, 4, 0)]))
    for m in (0, 1):                        # slot0's rb8 pair, 256 wide
        sched.append((m, 4, B, 256, [(0, B, 4, 0)]))
    return sched


SCHED = _schedule()
NRS = len(SCHED)           # 18 rowsum slots
CE_COLS = 4
ORS = NRS + CE_COLS        # colsum block offset in the out tensor
IDX_YP = 8                 # emit the yp DMA before this sim group
IDX_CE0 = 12               # emit CE group t after sim group IDX_CE0+t
NWARM = 16                 # cold N=256 warm-up matmuls (~3.4us at 1.2GHz)


def _build_kernel(tc, xt, yp, sel, out):
    """Per-core Tile kernel.

    xt:  [P, 5*KT*2*512]  fp8  streamed cols (rotated by 512c), chunk-major,
                               chunk layout [KT, 2, 512] per partition;
                               value = S8*xn[abscol, k*256+i*128+p].
                               Chunk 0 doubles as the stationary operand.
    yp:  [P, 4*C]         bf16 this core's y_pred rows (512c..512c+512)
    sel: [P, 16*16]       bf16 one-hot: sel[p, 16*j + r] = (r == j)
    out: [P, ORS+W]       f32  rowsums | CE sums | colsum rows 0..15
    """
    nc = tc.nc
    with ExitStack() as ctx:
        pers = ctx.enter_context(tc.tile_pool(name="pers", bufs=1))
        epool = ctx.enter_context(tc.tile_pool(name="epool", bufs=NRS + 1))
        cepool = ctx.enter_context(tc.tile_pool(name="cepool", bufs=2))
        psum = ctx.enter_context(
            tc.tile_pool(name="psum", bufs=4, space=bass.MemorySpace.PSUM)
        )
        cspsum = ctx.enter_context(
            tc.tile_pool(name="cspsum", bufs=1, space=bass.MemorySpace.PSUM)
        )

        XT = [
            pers.tile([P, KT, 2, W], _FP8, name=f"xt{h}", tag=f"xt{h}")
            for h in range(NCH)
        ]
        YPB = pers.tile([P, 4 * C], _BF16)
        SEL = pers.tile([P, 16, 16], _BF16)
        OUTSB = pers.tile([P, ORS + W], _F32)
        bias_s = pers.tile([P, 1], _F32)       # 0.5*TAU for the sim affine
        bias_z = pers.tile([P, 1], _F32)       # 0.0 for plain exp
        warm = pers.tile([P, 1], _F32)
        ZW = pers.tile([P, 256], _BF16)        # zeros, PE warm-up operand

        # constants on the (otherwise idle until mid-stream) vector engine:
        # its dispatch is fast, so the PE warm-up starts almost immediately
        nc.vector.memset(ZW[:], 0.0)
        nc.vector.memset(bias_s[:], 0.5 * TAU)
        nc.vector.memset(bias_z[:], 0.0)
        # warm the exp table (ACT_TABLE_LOAD ~1.3us) before any data lands
        nc.scalar.activation(warm[:], bias_z[:], _EXP, bias=bias_z[:],
                             scale=1.0)

        # PE warm-up: dummy matmuls covering the input-DMA latency; they flip
        # the HAM clock gate to 8/8 so the real stream runs at 2.4GHz
        wps = psum.tile([P, W], _F32, tag="ps")
        for _ in range(NWARM):
            nc.tensor.matmul(wps[:, 0:256], ZW[:, 0:P], ZW[:], start=True,
                             stop=True)

        # --- input DMAs (sync queue, consumption order; each chunk lands as
        # one contiguous 4KB run per partition -> 128 descriptors) ---
        span = KT * 2 * W
        for h in range(NCH):
            nc.sync.dma_start(
                XT[h][:],
                xt[:, h * span:(h + 1) * span].rearrange(
                    "p (k i n) -> p k i n", k=KT, i=2),
            )
        nc.scalar.dma_start(SEL[:], sel.rearrange("p (j r) -> p j r", j=16))

        act_scale = 0.5 * TAU / (S8 * S8)
        ets = []               # (et tile, cs_spans), for the deferred colsums

        def sim_group(idx, m, ch, rhs_off, width, cs_spans):
            ps = psum.tile([P, W], _F32, tag="ps")
            for k in range(KT):
                nc.tensor.matmul(
                    ps[:, 0:width],
                    XT[0][:, k, :, m * P:(m + 1) * P],
                    XT[ch][:, k, :, rhs_off:rhs_off + width],
                    start=(k == 0),
                    stop=(k == KT - 1),
                    perf_mode=_DR,
                )
            et = epool.tile([P, W], _BF16)
            nc.scalar.activation(et[:, 0:width], ps[:, 0:width], _EXP,
                                 bias=bias_s[:], scale=act_scale)
            nc.vector.reduce_sum(OUTSB[:, idx:idx + 1], et[:, 0:width],
                                 axis=_AXX)
            ets.append((et, cs_spans))

        # CE sits mid-stream: by then the yp DMA (issued at IDX_YP) has
        # landed, so the scalar queue never stalls on it
        def ce_group(t):
            ece = cepool.tile([P, W], _BF16)
            nc.scalar.activation(ece[:], YPB[:, t * C:(t + 1) * C], _EXP,
                                 bias=bias_z[:], scale=1.0)
            nc.vector.reduce_sum(OUTSB[:, NRS + t:NRS + t + 1], ece[:],
                                 axis=_AXX)

        for idx, (m, ch, rhs_off, width, cs_spans) in enumerate(SCHED):
            if idx == IDX_YP:
                nc.scalar.dma_start(
                    YPB[:], yp.rearrange("p (t c) -> p t c", t=4))
            sim_group(idx, m, ch, rhs_off, width, cs_spans)
            if IDX_CE0 <= idx < IDX_CE0 + CE_COLS:
                ce_group(idx - IDX_CE0)

        # rowsums/CE are complete as soon as the reduces drain -> ship them
        # while the colsum matmuls still run
        nc.scalar.dma_start(out[:, 0:ORS], OUTSB[:, 0:ORS])

        # deferred colsums: one-hot selector matmuls, back-to-back on the PE
        # after the last sim group.  Each DR<->normal mode switch costs
        # ~250ns of lost LDWEIGHTS overlap, so these must NOT interleave
        # with the sim stream: tile_wait_until pins them past the end of the
        # scheduler's virtual timeline (compile-time ordering only — the
        # runtime just executes the final list as dependencies allow).
        # start=True clears accumulation state at BANK granularity -> only
        # the first matmul carries it.
        CS = cspsum.tile([16, W], _F32)
        n_cs = sum(len(cs) for _, cs in ets)
        i = 0
        with tc.tile_wait_until(0.1):
            for et, cs_spans in ets:
                for lo, hi, row, dst in cs_spans:
                    nc.tensor.matmul(
                        CS[:, dst:dst + (hi - lo)],
                        SEL[:, row, :],
                        et[:, lo:hi],
                        start=(i == 0),
                        stop=(i == n_cs - 1),
                    )
                    i += 1
        nc.vector.tensor_copy(OUTSB[0:16, ORS:], CS[:])
        nc.scalar.dma_start(out[0:16, ORS:], OUTSB[0:16, ORS:])


_NC_CACHE = None


def _get_nc():
    global _NC_CACHE
    if _NC_CACHE is None:
        nc = bacc.Bacc(
            "TRN2", target_bir_lowering=False, debug=False,
            enable_asserts=False, num_devices=NCORES,
        )
        xt_d = nc.dram_tensor("xt", [P, NCH * KT * 2 * W], _FP8,
                              kind="ExternalInput")
        yp_d = nc.dram_tensor("yp", [P, 4 * C], _BF16, kind="ExternalInput")
        sel_d = nc.dram_tensor("sel", [P, 16 * 16], _BF16,
                               kind="ExternalInput")
        out_d = nc.dram_tensor("out", [P, ORS + W], _F32,
                               kind="ExternalOutput")
        with tile.TileContext(nc) as tc:
            _build_kernel(tc, xt_d.ap(), yp_d.ap(), sel_d.ap(), out_d.ap())
        nc.compile()
        _NC_CACHE = nc
    return _NC_CACHE


def _pack_dr(zT, cols_idx):
    """zT [D, N] f32 -> [P, KT, 2, L] fp8 with the DoubleRow pairing:
    partition p, (k, i, n) <-> contraction k*256 + i*128 + p, col cols_idx[n].
    """
    fp8np = mybir.dt.np(_FP8)
    sub = zT[:, cols_idx]                               # [D, L]
    L = sub.shape[1]
    q = sub.reshape(KT, 2, P, L).transpose(2, 0, 1, 3)  # [P, KT, 2, L]
    return np.ascontiguousarray(q.astype(fp8np))


def _run_device(xnT, y_pred, trace=False):
    zT = (xnT * S8).astype(np.float32)  # [D, N], pre-scaled
    bf16np = mybir.dt.np(_BF16)
    sel = np.zeros((P, 16, 16), np.float32)
    for j in range(16):
        sel[:, j, j] = 1.0
    sel = sel.astype(bf16np).reshape(P, 256)

    in_maps = []
    for c in range(NCORES):
        base = W * c
        chunk_cols = [np.arange(h * W, (h + 1) * W) for h in range(4)]
        chunk_cols.append(np.concatenate(
            [np.arange(B, 2 * B), np.arange(2048, 2048 + B)]))
        parts = [
            _pack_dr(zT, (cols + base) % N).reshape(P, KT * 2 * W)
            for cols in chunk_cols
        ]
        xt8 = np.ascontiguousarray(np.concatenate(parts, axis=1))
        ypb = (
            np.ascontiguousarray(y_pred[base:base + W])
            .reshape(4, P, C).transpose(1, 0, 2).reshape(P, 4 * C)
            .astype(bf16np)
        )
        in_maps.append({"xt": xt8, "yp": ypb, "sel": sel})
    res = run_bass_kernel_spmd(
        _get_nc(), in_maps, core_ids=list(range(NCORES)), trace=trace,
    )

    T = np.zeros(N, np.float64)
    R = np.empty(N, np.float64)
    for c, r in enumerate(res.results):
        o = r["out"].astype(np.float64)                 # [P, ORS+W]
        base = W * c
        for idx, (m, ch, rhs_off, width, cs_spans) in enumerate(SCHED):
            rows = slice(base + m * P, base + (m + 1) * P)
            T[rows] += o[:, idx]
        for t in range(4):
            R[base + t * P:base + (t + 1) * P] = o[:, NRS + t]
        # colsum rows: CS[j, w] = this core's column sums at stream col
        # 512*j + w (j<4) / 2048 + w (j=4, w<256).  Unwritten regions
        # (diagonal skips): CS[0, 0:256], CS[4, 256:512], rows 5..15.
        cs = o[0:16, ORS:]                              # [16, W]
        cs[0, 0:B] = 0.0
        for j, (off, wd) in enumerate([(0, W), (W, W), (2 * W, W),
                                       (3 * W, W), (4 * W, B)]):
            abscols = (np.arange(off, off + wd) + base) % N
            T[abscols] += cs[j, 0:wd]
    return T, R, res


def kernel(layer_embeds, y_true, y_pred):
    x = np.asarray(layer_embeds, dtype=np.float32)
    yt = np.asarray(y_true).astype(np.int64)
    yp = np.asarray(y_pred, dtype=np.float32)

    # normalize rows (torch-style eps clip)
    norms = np.maximum(
        np.sqrt((x.astype(np.float64) ** 2).sum(1, keepdims=True)), 1e-8
    )
    xn = (x / norms).astype(np.float32)
    xnT = np.ascontiguousarray(xn.T)  # [D, N]

    trace = bool(int(os.environ.get("CLCE_TRACE", "0")))
    T, R, res = _run_device(xnT, yp, trace=trace)
    if trace:
        kernel.last_results = res

    # --- host-side small terms (O(N * class_size)) ---
    fp8np = mybir.dt.np(_FP8)
    xq = (xn * S8).astype(fp8np).astype(np.float64) / S8  # device-visible xn
    counts = np.bincount(yt, minlength=C)
    P_ = np.zeros(N, np.float64)
    slot0 = np.zeros(N, np.float64)
    for cval in np.unique(yt):
        idx = np.where(yt == cval)[0]
        subq = xq[idx]
        sq = (subq @ subq.T + 1.0) * (0.5 * TAU)   # device-matching sim
        P_[idx] = np.exp(sq).sum(1)
        if len(idx) >= 2:
            # slot0 feeds the final formula directly -> use full precision
            sub = xn[idx].astype(np.float64)
            s = (sub @ sub.T + 1.0) * (0.5 * TAU)
            firstpos = np.where(np.arange(len(idx)) == 0, 1, 0)
            slot0[idx] = s[np.arange(len(idx)), firstpos]

    num_neg = N - counts[yt]
    S = T - P_
    Z = (2 * N - 2 - num_neg).astype(np.float64)
    cl = (np.log(np.exp(slot0) + S + Z) - slot0).mean()
    ce = (
        np.log(R) - yp[np.arange(N), yt].astype(np.float64)
    ).mean()
    loss = LAMBD * cl + (1.0 - LAMBD) * ce
    return np.asarray(loss, dtype=np.float32)
